# revision 1
# baseline (speedup 1.0000x reference)
"""DenseTransformerConv (GNN message passing) fused Bass/Tile kernel for Trainium2.

Sharding: 8 cores = 4 batches x 2 i-halves (data parallel; weights replicated).
Per core: b = core//2, destination-node block i in [128*(core%2), +128).

Math per core (H=8 heads, C=32, DE=64), with 1/sqrt(C) pre-folded into Wk/bk
and We^T on the host:
  Q  = x_i @ Wq + bq            (kept transposed: d'-partitions)
  Ks = (x @ Wk + bk)/sqrt(C)    (transposed)
  V  = x @ Wv + bv              (natural: n-partitions)
  u[i,h,:] = (We_h/sqrt(C))^T Q[i,h,:]      (edge projection folded into Q)
  s[i,j,h] = Q.Ks^T + edge[i,j,:].u[i,h,:]  (j-partitioned score tiles)
  a = exp(s) * mask                          (scores are O(1); no max pass)
  agg[i,h,:] = sum_j a[i,j,h] * edge[i,j,:]  (edge stationary, j-contraction)
  out_attn[i,h,:] = (sum_j a.V_h + agg @ We_h) / sum_j a
  out = out_attn + x_i @ Ws + bs

Scores are computed j-PARTITIONED so the softmax sum over j is a ones-matmul
on PE and alpha feeds the aggregation/out_v matmuls with no transpose.  The
de-contraction (edge score) uses PE pair-transposed (2i*64de, 128j) bf16
tiles built on the fly from the natural-layout edge copy.
"""

import sys

for _p in ("/opt/trn_rl_repo",):
    if _p not in sys.path:
        sys.path.append(_p)

import numpy as np

B, N, D, DE, H, C = 4, 256, 256, 64, 8, 32
P = 128
NCORES = 8

_PROGRAM = None


def _build_program():
    import concourse.bass as bass
    import concourse.mybir as mybir
    import concourse.tile as tile
    from concourse.bass import ds
    from concourse.masks import make_identity
    from contextlib import ExitStack

    f32 = mybir.dt.float32
    bf16 = mybir.dt.bfloat16
    u8 = mybir.dt.uint8
    AF = mybir.ActivationFunctionType
    MUL = mybir.AluOpType.mult
    ADD = mybir.AluOpType.add

    nc = bass.Bass()

    xT = nc.declare_dram_parameter("xT", [D, N], f32, isOutput=False)
    xTi = nc.declare_dram_parameter("xTi", [D, P], f32, isOutput=False)
    edge = nc.declare_dram_parameter("edge", [P, N, DE], f32, isOutput=False)
    maskT = nc.declare_dram_parameter("maskT", [N, P], u8, isOutput=False)
    Wq = nc.declare_dram_parameter("Wq", [D, D], f32, isOutput=False)
    Wks = nc.declare_dram_parameter("Wks", [D, D], f32, isOutput=False)
    Wv = nc.declare_dram_parameter("Wv", [D, D], f32, isOutput=False)
    Ws = nc.declare_dram_parameter("Ws", [D, D], f32, isOutput=False)
    WeTs = nc.declare_dram_parameter("WeTs", [D, DE], f32, isOutput=False)
    We = nc.declare_dram_parameter("We", [DE, D], f32, isOutput=False)
    bq = nc.declare_dram_parameter("bq", [D], f32, isOutput=False)
    bks = nc.declare_dram_parameter("bks", [D], f32, isOutput=False)
    bv = nc.declare_dram_parameter("bv", [D], f32, isOutput=False)
    bs = nc.declare_dram_parameter("bs", [D], f32, isOutput=False)
    out = nc.declare_dram_parameter("out", [P, D], f32, isOutput=True)

    with tile.TileContext(nc) as tc, ExitStack() as ctx:
        singles = ctx.enter_context(tc.tile_pool(name="singles", bufs=1))
        eT_sb_pool = ctx.enter_context(tc.tile_pool(name="eT_sb", bufs=3))
        fin_pool = ctx.enter_context(tc.tile_pool(name="fin", bufs=2))
        # persistent PSUM pools (stack-allocated: 1 + 2 + 1 = 4 banks)
        f_pool = ctx.enter_context(tc.tile_pool(name="f", bufs=1, space="PSUM"))
        ov_pool = ctx.enter_context(tc.tile_pool(name="ov", bufs=1, space="PSUM"))
        agg_pool = ctx.enter_context(tc.tile_pool(name="agg", bufs=1, space="PSUM"))
        # scoped PSUM pools entered/exited around each phase
        proj_ctx = ExitStack()
        proj_ps = proj_ctx.enter_context(
            tc.tile_pool(name="proj_ps", bufs=1, space="PSUM")
        )

        # ---------------- constants / weights prologue ----------------
        ident_bf = singles.tile([P, P], bf16)
        make_identity(nc, ident_bf)
        ident_f32 = singles.tile([P, P], f32)
        make_identity(nc, ident_f32)
        ones_bf = singles.tile([P, P], bf16)
        nc.vector.memset(ones_bf, 1.0)
        zrow = singles.tile([1, 512], bf16)
        nc.vector.memset(zrow, 0.0)

        def mm(out_ap, lhsT, rhs, **kw):
            # All PSUM regions are zeroed by an explicit start=True matmul, so
            # accumulation order is irrelevant; skip the sim's group tracking
            # (stop_tensor_calc is sim-bookkeeping only, not a HW effect).
            kw.setdefault("skip_group_check", True)
            nc.tensor.matmul(out_ap, lhsT, rhs, **kw)

        def zero_mm(out_ap, m, n):
            # start=True matmul writing 0 to every byte of a PSUM bank region;
            # makes all subsequent accumulating matmuls order-independent.
            mm(out_ap, zrow[:, :m], zrow[:, :n], start=True, stop=False)

        def load_w2(src):  # (D, D) f32 -> [128, 2, 256] bf16 (row-chunked)
            t = singles.tile([P, 2, D], bf16, tag=f"w_{src.name}")
            for kc in range(2):
                nc.gpsimd.dma_start(out=t[:, kc, :], in_=src[ds(kc * P, P), :])
            return t

        Wq_sb = load_w2(Wq)
        Wks_sb = load_w2(Wks)
        Wv_sb = load_w2(Wv)
        Ws_sb = load_w2(Ws)

        # WeTs head-split: [32c, 8h, 64de] (keeps every PE operand at base 0)
        WeTs_sb = singles.tile([32, H, DE], bf16)
        nc.gpsimd.dma_start(
            out=WeTs_sb,
            in_=bass.AP(
                tensor=WeTs[:, :].tensor,
                offset=0,
                ap=[[DE, 32], [32 * DE, H], [1, DE]],
            ),
        )
        We_sb = singles.tile([DE, D], bf16)
        nc.gpsimd.dma_start(out=We_sb, in_=We[:, :])

        xT_sb = singles.tile([P, 2, N], bf16)
        xTi_sb = singles.tile([P, 2, P], bf16)
        for kc in range(2):
            nc.gpsimd.dma_start(out=xT_sb[:, kc, :], in_=xT[ds(kc * P, P), :])
            nc.gpsimd.dma_start(out=xTi_sb[:, kc, :], in_=xTi[ds(kc * P, P), :])

        ones_row = singles.tile([1, 512], bf16)
        nc.vector.memset(ones_row, 1.0)

        def load_brow(src):  # (256,) f32 -> (1, 256) bf16 row for rank-1 bias MMs
            t = singles.tile([1, D], bf16, tag=f"b_{src.name}", name=f"br_{src.name}")
            nc.gpsimd.dma_start(out=t, in_=src[:])
            return t

        bq_row = load_brow(bq)
        bks_row = load_brow(bks)
        bv_row = load_brow(bv)
        bs_row = load_brow(bs)

        mT_u8 = singles.tile([P, 2, P], u8)
        mask_bias = singles.tile([P, 2, P], f32)
        for cj in range(2):
            nc.sync.dma_start(out=mT_u8[:, cj, :], in_=maskT[ds(cj * P, P), :])
            # (mask * 50) - 50: 0 for allowed edges, -50 for masked ones
            nc.vector.tensor_scalar(
                out=mask_bias[:, cj, :],
                in0=mT_u8[:, cj, :],
                scalar1=50.0,
                scalar2=50.0,
                op0=MUL,
                op1=mybir.AluOpType.subtract,
            )

        # ---------------- projections (head-split: [32c, 8h, n]) ----------------
        QTi_sb = singles.tile([32, H, P], bf16)
        q_ps = proj_ps.tile([32, H * P], f32, tag="proj")
        for bank in range(2):
            zero_mm(q_ps[:, ds(bank * 512, 512)], 32, 512)
        for h in range(H):
            for kc in range(2):
                mm(
                    q_ps[:, ds(h * P, P)],
                    Wq_sb[:, kc, ds(h * 32, 32)],
                    xTi_sb[:, kc, :],
                    start=False,
                    stop=(kc == 1 and h % 4 == 3),
                )
        for h in range(H):
            mm(
                q_ps[:, ds(h * P, P)],
                bq_row[:, ds(h * 32, 32)],
                ones_row[:, :P],
                start=False,
                stop=False,
            )
        for h in range(H):
            nc.scalar.activation(
                out=QTi_sb[:, h, :], in_=q_ps[:, ds(h * P, P)], func=AF.Copy
            )

        KT_sb = singles.tile([32, H, N], bf16)  # pre-scaled by 1/sqrt(C)
        for hh in range(2):
            k_ps = proj_ps.tile([32, 4 * N], f32, tag="proj")
            for bank in range(2):
                zero_mm(k_ps[:, ds(bank * 512, 512)], 32, 512)
            for h4 in range(4):
                h = hh * 4 + h4
                for kc in range(2):
                    mm(
                        k_ps[:, ds(h4 * N, N)],
                        Wks_sb[:, kc, ds(h * 32, 32)],
                        xT_sb[:, kc, :],
                        start=False,
                        stop=(kc == 1 and h4 % 2 == 1),
                    )
            for h4 in range(4):
                h = hh * 4 + h4
                mm(
                    k_ps[:, ds(h4 * N, N)],
                    bks_row[:, ds(h * 32, 32)],
                    ones_row[:, :N],
                    start=False,
                    stop=False,
                )
            for h4 in range(4):
                h = hh * 4 + h4
                nc.scalar.activation(
                    out=KT_sb[:, h, :], in_=k_ps[:, ds(h4 * N, N)], func=AF.Copy
                )

        V_sb = singles.tile([P, 2, D], bf16)  # (n-part, d')
        for nc2 in range(2):
            v_ps = proj_ps.tile([P, D], f32, tag="proj")
            zero_mm(v_ps, P, D)
            for kc in range(2):
                mm(
                    v_ps,
                    xT_sb[:, kc, ds(nc2 * P, P)],
                    Wv_sb[:, kc, :],
                    start=False,
                    stop=(kc == 1),
                )
            mm(v_ps, ones_row[:, :P], bv_row, start=False, stop=False)
            nc.scalar.activation(out=V_sb[:, nc2, :], in_=v_ps, func=AF.Copy)

        # skip connection -> F psum (stays open until the epilogue transposes)
        F_ps = f_pool.tile([P, D], f32)
        zero_mm(F_ps, P, D)
        for kc in range(2):
            mm(
                F_ps, xTi_sb[:, kc, :], Ws_sb[:, kc, :], start=False, stop=False
            )

        # QK scores, computed once per j-chunk: (128j, 8h, 128i)
        qk_sb = singles.tile([P, 2, H, P], f32)
        for cj in range(2):
            qk_ps = proj_ps.tile([P, H * P], f32, tag="proj")
            for bank in range(2):
                zero_mm(qk_ps[:, ds(bank * 512, 512)], P, 512)
            for h in range(H):
                mm(
                    qk_ps[:, ds(h * P, P)],
                    KT_sb[:, h, ds(cj * P, P)],
                    QTi_sb[:, h, :],
                    start=False,
                    stop=(h % 4 == 3),
                )
            nc.scalar.activation(out=qk_sb[:, cj, :, :], in_=qk_ps, func=AF.Copy)
            # fold the additive mask (broadcast over heads)
            nc.vector.tensor_tensor(
                out=qk_sb[:, cj, :, :],
                in0=qk_sb[:, cj, :, :],
                in1=mask_bias[:, cj, :].unsqueeze(1).broadcast_to([P, H, P]),
                op=ADD,
            )

        # u: per head uT_h = WeTs_h^T Q_h -> (64de, 128i) into one psum tile
        u_ps = proj_ps.tile([DE, H * P], f32, tag="u", bufs=1)
        for bank in range(2):
            zero_mm(u_ps[:, ds(bank * 512, 512)], DE, 512)
        for h in range(H):
            mm(
                u_ps[:, ds(h * P, P)],
                WeTs_sb[:, h, :],
                QTi_sb[:, h, :],
                start=False,
                stop=(h % 4 == 3),
            )
        # scatter into block-diag u_blk [128=(ii,de), 64 pairs, 16=(ii,h)] bf16
        u_blk = singles.tile([P, 64, 16], bf16)
        nc.gpsimd.memset(u_blk, 0.0)
        u_ps_v = u_ps.rearrange("p (h pair ii) -> p h pair ii", h=H, ii=2)
        for h in range(H):
            for ii in range(2):
                nc.vector.tensor_copy(
                    out=u_blk[ds(ii * DE, DE), :, ii * 8 + h],
                    in_=u_ps_v[:, h, :, ii],
                )

        proj_ctx.close()  # release prologue PSUM banks
        stream_ctx = ExitStack()
        qe_pool = stream_ctx.enter_context(
            tc.tile_pool(name="qe", bufs=2, space="PSUM")
        )
        eT_ps_pool = stream_ctx.enter_context(
            tc.tile_pool(name="eT_ps", bufs=2, space="PSUM")
        )

        import os as _os
        _BI = int(_os.environ.get("BISECT", "0"))

        def _emit_out(src_ap):
            t_dbg = singles.tile([P, D], f32, tag="dbg", name="dbg_out")
            nc.vector.memset(t_dbg, 0.0)
            pp = src_ap.partition_size()
            nc.vector.tensor_copy(
                out=t_dbg[ds(0, pp), : src_ap.shape[-1]], in_=src_ap
            )
            nc.sync.dma_start(out=out[:, :], in_=t_dbg)

        if _BI == 1:
            _emit_out(qk_sb[:, 0, 0, :])
            proj_ctx.close()
            return nc

        # ---------------- edge stream ----------------
        # natural-layout edge: per (iq, cj) one [128j, 32i, 64de] bf16 tile
        en_t = {}
        for iq in range(4):
            for cj in range(2):
                t = singles.tile([P, 32, DE], bf16, tag=f"en_{iq}_{cj}")
                nc.gpsimd.dma_start(
                    out=t,
                    in_=edge[ds(iq * 32, 32), ds(cj * P, P), :].rearrange(
                        "i j de -> j i de"
                    ),
                )
                en_t[(iq, cj)] = t

        alpha_t = {}
        # output accumulator: two 1-bank tiles; h -> (tile (h%4)//2, pbase
        # 32*(h%2), free offset 128*(h//4)).  Avoids PE partition base 96.
        OV_t = [
            ov_pool.tile([64, 2 * P], f32, tag="ov0", name="ov0"),
            ov_pool.tile([64, 2 * P], f32, tag="ov1", name="ov1"),
        ]
        for t in range(2):
            zero_mm(OV_t[t], 64, 2 * P)
        agg_sb = [None, None]
        agg_ps = None

        for iq in range(4):
            half = iq // 2
            for cj in range(2):
                # score tile: [128j, 8h, 32i] f32 (one PSUM bank)
                qe = qe_pool.tile([P, 32, H], f32)
                zero_mm(qe, P, 32 * H)
                # edge-score matmuls via pair-transposed tiles
                for tq in range(4):  # 4 pairs per eT tile, 16 pairs per iq
                    eT_ps = eT_ps_pool.tile([P, 512], bf16)
                    for p4 in range(4):
                        pr = iq * 16 + tq * 4 + p4  # global pair 0..63
                        li = pr * 2 - iq * 32  # local i in [0, 32)
                        mm(
                            eT_ps[:, ds(p4 * P, P)],
                            en_t[(iq, cj)][:, ds(li, 2), :],
                            ident_bf,
                            is_transpose=True,
                            start=True,
                            stop=True,
                        )
                    eT_sb = eT_sb_pool.tile([P, 512], bf16)
                    if tq % 2 == 0:
                        nc.scalar.activation(out=eT_sb, in_=eT_ps, func=AF.Copy)
                    else:
                        nc.vector.tensor_copy(out=eT_sb, in_=eT_ps)
                    for p4 in range(4):
                        pr = iq * 16 + tq * 4 + p4
                        li = pr * 2 - iq * 32
                        mm(
                            qe[:, ds(li, 2), :],
                            eT_sb[:, ds(p4 * P, P)],
                            u_blk[:, pr, :],
                            start=False,
                            stop=(tq == 3 and p4 == 3),
                        )

                # exp + mask -> unnormalized alpha (bf16)
                if _BI == 2 and iq == 0 and cj == 0:
                    _emit_out(qe[:, :, 0])
                    stream_ctx.close()
                    proj_ctx.close()
                    return nc
                al = singles.tile([P, 32, H], bf16, tag=f"al_{cj}_{iq}")
                alpha_t[(cj, iq)] = al
                s_sum = fin_pool.tile([P, 32, H], f32, tag="s_sum")
                nc.vector.tensor_tensor(
                    out=s_sum,
                    in0=qe,
                    in1=qk_sb[:, cj, :, ds(iq * 32, 32)].rearrange(
                        "p h i -> p i h"
                    ),
                    op=ADD,
                )
                nc.scalar.activation(out=al, in_=s_sum, func=AF.Exp)

            if _BI == 3 and iq == 0:
                _emit_out(alpha_t[(0, 0)][:, :, 0])
                stream_ctx.close()
                proj_ctx.close()
                return nc
            # aggregation (j-contraction, edge stationary) for this iq
            if iq % 2 == 0:
                agg_ps = agg_pool.tile([DE, 64 * 8], f32)
                zero_mm(agg_ps, DE, 512)
            for cj in range(2):
                al0 = alpha_t[(cj, iq)]
                for il in range(32):
                    i = iq * 32 + il
                    mm(
                        agg_ps[:, ds((i - half * 64) * 8, 8)],
                        en_t[(iq, cj)][:, il, :],
                        al0[:, il, :],
                        start=False,
                        stop=(cj == 1 and il == 31 and iq % 2 == 1),
                    )
            # out_v for this iq
            for cj in range(2):
                al0 = alpha_t[(cj, iq)]
                for h in range(H):
                    mm(
                        OV_t[(h % 4) // 2][
                            ds(32 * (h % 2), 32), ds(P * (h // 4) + iq * 32, 32)
                        ],
                        V_sb[:, cj, ds(h * 32, 32)],
                        al0[:, :, h],
                        start=False,
                        stop=False,
                    )
            if iq % 2 == 1:
                t = singles.tile([DE, 64, 8], bf16, tag=f"aggsb_{half}")
                nc.scalar.activation(out=t, in_=agg_ps, func=AF.Copy)
                agg_sb[half] = t

        if _BI == 4:
            _emit_out(agg_sb[1][:, :, 0])
            stream_ctx.close()
            proj_ctx.close()
            return nc

        # ---------------- epilogue ----------------
        # out_e: OV_h += We_h^T-projected aggregation
        for h in range(H):
            for half in range(2):
                mm(
                    OV_t[(h % 4) // 2][
                        ds(32 * (h % 2), 32), ds(P * (h // 4) + half * 64, 64)
                    ],
                    We_sb[:, ds(h * 32, 32)],
                    agg_sb[half][:, :, h],
                    start=False,
                    stop=(h >= 4 and half == 1),
                )

        if _BI == 6:
            # skip out_e: only test den + epilogue paths
            pass
        stream_ctx.close()  # release qe/eT PSUM banks
        den_ctx = ExitStack()
        den_pool = den_ctx.enter_context(
            tc.tile_pool(name="den", bufs=1, space="PSUM")
        )

        # denominators: ones-matmul over alpha (replicated across partitions)
        den_ps = []
        for hhalf in range(2):
            den_ps.append(
                den_pool.tile(
                    [P, P, 4], f32, tag=f"den{hhalf}", bufs=1, name=f"den{hhalf}"
                )
            )
        for hhalf in range(2):
            zero_mm(den_ps[hhalf], P, 512)
        for hhalf in range(2):
            for iq in range(4):
                for cj in range(2):
                    mm(
                        den_ps[hhalf][:, ds(iq * 32, 32), :],
                        ones_bf,
                        alpha_t[(cj, iq)][:, :, ds(hhalf * 4, 4)],
                        start=False,
                        stop=(cj == 1 and iq == 3),
                    )
        if _BI == 7:
            _emit_out(den_ps[0][:, :, 0])
            den_ctx.close()
            proj_ctx.close()
            return nc
        den_sb = singles.tile([P, P, H], f32)
        for hhalf in range(2):
            nc.vector.tensor_scalar_add(
                out=den_sb[:, :, ds(hhalf * 4, 4)], in0=den_ps[hhalf], scalar1=1e-30
            )
        nc.vector.reciprocal(out=den_sb, in_=den_sb)

        if _BI == 8:
            _emit_out(den_sb[:, :, 0])
            den_ctx.close()
            proj_ctx.close()
            return nc
        # normalize, transpose into a fresh PSUM tile (transpose matmuls
        # cannot accumulate onto prior PSUM contents on HW), then combine
        T_ps = den_pool.tile([P, D], f32, bufs=1)
        for h in range(H):
            po = 32 * (h % 2)
            fo = P * (h // 4)
            ovn = fin_pool.tile([P, P], f32, tag="ovn")
            nc.vector.tensor_tensor(
                out=ovn[ds(po, 32), :],
                in0=OV_t[(h % 4) // 2][ds(po, 32), ds(fo, P)],
                in1=den_sb[ds(po, 32), :, h],
                op=MUL,
            )
            mm(
                T_ps[:, ds(h * 32, 32)],
                ovn[ds(po, 32), :],
                ident_f32[ds(po, 32), ds(po, 32)],
                is_transpose=True,
                start=True,
                stop=True,
            )

        mm(F_ps, ones_row[:, :P], bs_row, start=False, stop=True)
        t_sb = singles.tile([P, D], f32)
        nc.scalar.activation(out=t_sb, in_=T_ps, func=AF.Copy)
        outp = singles.tile([P, D], f32)
        nc.vector.tensor_tensor(out=outp, in0=F_ps, in1=t_sb, op=ADD)
        nc.sync.dma_start(out=out[:, :], in_=outp)
        den_ctx.close()

    return nc


def _split_multi_waits(nc):
    """Walrus TRN2 codegen encodes at most ONE sync wait per engine
    instruction; Tile's wait assignment is not transitively minimal and
    emits 2-3.  Hoist all but one wait onto same-engine no-ops."""
    import concourse.mybir as mybir

    for fn in nc.m.functions:
        for blk in fn.blocks:
            new_insts = []
            for inst in blk.instructions:
                si = inst.sync_info
                if (
                    si is not None
                    and len(si.on_wait) > 1
                    and type(inst).__name__ != "InstEventSemaphore"
                ):
                    waits = list(si.on_wait)
                    for k, w in enumerate(waits[:-1]):
                        nop = mybir.InstNoOp(name=f"{inst.name}-sw{k}", ins=[], outs=[])
                        nop.engine = inst.engine
                        nop.sync_info = mybir.SyncInfo(on_wait=[w], on_update=[])
                        nc.register_instruction(nop)
                        new_insts.append(nop)
                    inst.sync_info = mybir.SyncInfo(
                        on_wait=[waits[-1]], on_update=list(si.on_update)
                    )
                new_insts.append(inst)
            blk.instructions = new_insts


def _get_program():
    global _PROGRAM
    if _PROGRAM is None:
        nc = _build_program()
        _split_multi_waits(nc)
        _PROGRAM = nc
    return _PROGRAM


def _prep_core_inputs(c, x, edge_attr, attn_mask, ws):
    b, ih = c // 2, c % 2
    i0 = ih * P
    return {
        "xT": np.ascontiguousarray(x[b].T),
        "xTi": np.ascontiguousarray(x[b, i0 : i0 + P].T),
        "edge": np.ascontiguousarray(edge_attr[b, i0 : i0 + P]),
        "maskT": np.ascontiguousarray(attn_mask[b, i0 : i0 + P].T.astype(np.uint8)),
        **ws,
    }


def kernel(x, edge_attr, attn_mask, W_q, b_q, W_k, b_k, W_v, b_v, W_e, W_s, b_s):
    from concourse.bass_utils import run_bass_kernel_spmd

    x = np.asarray(x, dtype=np.float32)
    edge_attr = np.asarray(edge_attr, dtype=np.float32)
    attn_mask = np.asarray(attn_mask)
    scale = np.float32(1.0 / np.sqrt(C))
    ws = {
        "Wq": np.ascontiguousarray(np.asarray(W_q, np.float32)),
        "Wks": np.ascontiguousarray(np.asarray(W_k, np.float32) * scale),
        "Wv": np.ascontiguousarray(np.asarray(W_v, np.float32)),
        "Ws": np.ascontiguousarray(np.asarray(W_s, np.float32)),
        "WeTs": np.ascontiguousarray(np.asarray(W_e, np.float32).T * scale),
        "We": np.ascontiguousarray(np.asarray(W_e, np.float32)),
        "bq": np.ascontiguousarray(np.asarray(b_q, np.float32)),
        "bks": np.ascontiguousarray(np.asarray(b_k, np.float32) * scale),
        "bv": np.ascontiguousarray(np.asarray(b_v, np.float32)),
        "bs": np.ascontiguousarray(np.asarray(b_s, np.float32)),
    }

    nc = _get_program()
    in_maps = [
        _prep_core_inputs(c, x, edge_attr, attn_mask, ws) for c in range(NCORES)
    ]
    res = run_bass_kernel_spmd(nc, in_maps, core_ids=list(range(NCORES)))
    outv = np.empty((B, N, D), dtype=np.float32)
    for c in range(NCORES):
        b, ih = c // 2, c % 2
        outv[b, ih * P : (ih + 1) * P] = np.asarray(res.results[c]["out"])
    return outv



# revision 21
# speedup vs baseline: 1.8563x; 1.8563x over previous
"""DenseTransformerConv (GNN message passing) fused Bass/Tile kernel for Trainium2.

Sharding: 8 cores = 4 batches x 2 i-halves (data parallel; weights replicated).
Per core: b = core//2, destination-node block i in [128*(core%2), +128).

v2 design (vs v1 baseline at ~110us):
  - Edge tensor arrives from the HOST in both layouts the PE needs:
      en  [cj][128 j, 128 i, 64 de]  bf16  (j-partitioned: agg / out_v)
      eT2 [cj][128 (ii,de), 64 pr, 128 j] fp8e4m3 (de-partitioned: scores)
    -> no on-chip PE transposes, fully contiguous >=1MiB HWDGE DMAs.
  - All small tensors (weights/x/mask-bias/biases/ones) packed into ONE
    [128, 4992] bf16 buffer -> a single DMA instead of ~20.
  - No zero-fill matmuls: every PSUM accumulation group opens with start=True.
  - Scores are j-partitioned (qk batched over i); the edge-score matmul
    uses the pair-transposed fp8 tiles as 128-col stationaries (FWL-able).
  - agg is pair-batched: lhsT = en[j, (2i,64de)] (128-col stationary),
    rhs = alpha[j, (2i,8h)]; the two off-diagonal blocks are junk and the
    diagonal is extracted with 4 strided DVE copies.
  - out_v/out_e accumulate I-PARTITIONED [128 i, 8h*33]: col 33 of each head
    is a ones-column of V, so the softmax denominator falls out of the same
    matmul; normalize+skip-add are two [128,256] DVE ops. No epilogue
    transposes, no 1024-element reciprocal.
  - Scores are scaled x16 on the host (Wq,bq) so u stays in fp8 range;
    exp() applies scale=1/16.
"""

import sys

for _p in ("/opt/trn_rl_repo",):
    if _p not in sys.path:
        sys.path.append(_p)

import numpy as np
import ml_dtypes

B, N, D, DE, H, C = 4, 256, 256, 64, 8, 32
P = 128
NCORES = 8

# packed buffer column offsets (bf16 elements)
OFF_WQ, OFF_WK, OFF_WV, OFF_WS = 0, 512, 1024, 1536
OFF_XT, OFF_XTI, OFF_WETS, OFF_WE = 2048, 2560, 2816, 3328
OFF_MB, OFF_BIAS, OFF_ONES = 3584, 3840, 4864
PACK_COLS = 5376

_PROGRAM = None


def _build_program():
    import concourse.bass as bass
    import concourse.mybir as mybir
    import concourse.tile as tile
    from concourse.bass import ds
    from contextlib import ExitStack

    f32 = mybir.dt.float32
    bf16 = mybir.dt.bfloat16
    fp8 = mybir.dt.float8e4
    AF = mybir.ActivationFunctionType
    MUL = mybir.AluOpType.mult
    ADD = mybir.AluOpType.add

    nc = bass.Bass()

    packed = nc.declare_dram_parameter("packed", [P, PACK_COLS], bf16, isOutput=False)
    en = nc.declare_dram_parameter("en", [2, P, P * DE], bf16, isOutput=False)
    eT2 = nc.declare_dram_parameter("eT2", [2, P, 64 * P], fp8, isOutput=False)
    out = nc.declare_dram_parameter("out", [P, D], f32, isOutput=True)

    with tile.TileContext(nc) as tc, ExitStack() as ctx:
        singles = ctx.enter_context(tc.tile_pool(name="singles", bufs=1))
        fin_pool = ctx.enter_context(tc.tile_pool(name="fin", bufs=2))
        # persistent PSUM: F (skip) 1 bank, OV_i 1 bank, agg 2 banks
        f_pool = ctx.enter_context(tc.tile_pool(name="f", bufs=1, space="PSUM"))
        ov_pool = ctx.enter_context(tc.tile_pool(name="ov", bufs=1, space="PSUM"))
        agg_pool = ctx.enter_context(tc.tile_pool(name="agg", bufs=1, space="PSUM"))
        proj_ctx = ExitStack()
        proj_ps = proj_ctx.enter_context(
            tc.tile_pool(name="proj_ps", bufs=2, space="PSUM")
        )

        def mm(out_ap, lhsT, rhs, **kw):
            # every PSUM region's first writer uses start=True; order of the
            # independent regions is irrelevant -> skip sim group tracking
            kw.setdefault("skip_group_check", True)
            nc.tensor.matmul(out_ap, lhsT, rhs, **kw)

        # ---------------- one DMA for everything small ----------------
        pk = singles.tile([P, PACK_COLS], bf16)
        nc.sync.dma_start(out=pk, in_=packed[:, :])

        def w_ap(base, kc, lo, n):  # weight chunk [128, n] cols lo..lo+n
            return pk[:, ds(base + kc * 256 + lo, n)]

        ones_row = pk[ds(0, 1), ds(OFF_ONES, 512)]

        def b_row(idx, lo, n):  # bias row [1, n]
            return pk[ds(0, 1), ds(OFF_BIAS + idx * 256 + lo, n)]

        # big edge DMAs on the sync (HWDGE) ring, eT2 first (scores unblock)
        eT2_sb = singles.tile([P, 2, 64 * P], fp8)
        en_sb = singles.tile([P, 2, P * DE], bf16)
        for cj in range(2):
            nc.sync.dma_start(out=eT2_sb[:, cj, :], in_=eT2[cj])
            nc.sync.dma_start(out=en_sb[:, cj, :], in_=en[cj])

        # ---------------- projections ----------------
        # head-split c-partitioned (PE base-partition must be 0/32/64):
        # QTi [32 c, 8 h, 128 i], KT [32 c, 8 h, 256 j] (pre-scaled)
        QTi = singles.tile([32, H, P], bf16)
        KT = singles.tile([32, H, N], bf16)
        q_ps = proj_ps.tile([32, H, P], f32, tag="proj")
        for h in range(H):
            for kc in range(2):
                mm(q_ps[:, h, :], w_ap(OFF_WQ, kc, h * 32, 32),
                   pk[:, ds(OFF_XTI + kc * 128, 128)],
                   start=(kc == 0), stop=False)
            mm(q_ps[:, h, :], b_row(0, h * 32, 32), ones_row[:, :P],
               start=False, stop=True)
        nc.scalar.activation(out=QTi, in_=q_ps, func=AF.Copy)
        for hh in range(2):
            k_ps = proj_ps.tile([32, 4, N], f32, tag="proj")
            for hm in range(4):
                h = hh * 4 + hm
                for kc in range(2):
                    mm(k_ps[:, hm, :], w_ap(OFF_WK, kc, h * 32, 32),
                       w_ap(OFF_XT, kc, 0, 256),
                       start=(kc == 0), stop=False)
                mm(k_ps[:, hm, :], b_row(1, h * 32, 32), ones_row[:, :N],
                   start=False, stop=True)
            nc.scalar.activation(out=KT[:, ds(hh * 4, 4), :], in_=k_ps,
                                 func=AF.Copy)

        # V [128 j, cj, 8 h, 33]: col 32 per head = 1.0 (denominator column)
        V_sb = singles.tile([P, 2, H, 33], bf16)
        nc.vector.memset(V_sb, 1.0)  # sets the ones-columns; rest overwritten
        for cj in range(2):
            v_ps = proj_ps.tile([P, D], f32, tag="proj")
            for kc in range(2):
                mm(v_ps, w_ap(OFF_XT, kc, cj * 128, 128),
                   w_ap(OFF_WV, kc, 0, 256),
                   start=(kc == 0), stop=False)
            mm(v_ps, ones_row[:, :P], b_row(2, 0, 256), start=False, stop=True)
            nc.vector.tensor_copy(
                out=V_sb[:, cj, :, 0:32],
                in_=v_ps.rearrange("p (h c) -> p h c", h=H),
            )

        # skip connection F = xTi^T @ Ws + bs  (i-partitioned, kept open)
        F_ps = f_pool.tile([P, D], f32)
        for kc in range(2):
            mm(F_ps, pk[:, ds(OFF_XTI + kc * 128, 128)], w_ap(OFF_WS, kc, 0, 256),
               start=(kc == 0), stop=False)
        mm(F_ps, ones_row[:, :P], b_row(3, 0, 256), start=False, stop=True)

        # QK scores + mask -> qk_sb [128 j, cj, 8 h, 128 i] bf16 (x16 scaled)
        qk_sb = singles.tile([P, 2, H, P], bf16)
        for cj in range(2):
            qk_ps = proj_ps.tile([P, H, P], f32, tag="proj")
            for h in range(H):
                mm(qk_ps[:, h, :], KT[:, h, ds(cj * 128, 128)],
                   QTi[:, h, :], start=True, stop=True)
            nc.vector.tensor_tensor(
                out=qk_sb[:, cj, :, :],
                in0=qk_ps,
                in1=pk[:, ds(OFF_MB + cj * 128, 128)]
                .unsqueeze(1).broadcast_to([P, H, P]),
                op=ADD,
            )

        # u_blk: block-diag fp8 [128 (ii,de), 64 pr, 16 (ii,h)]
        u_blk = singles.tile([P, 64, 16], fp8)
        nc.gpsimd.memset(u_blk, 0.0)
        for hh in range(2):
            u_ps = proj_ps.tile([DE, 4, P], f32, tag="proj")
            for hm in range(4):
                h = hh * 4 + hm
                mm(u_ps[:, hm, :], pk[ds(0, 32), ds(OFF_WETS + h * 64, 64)],
                   QTi[:, h, :], start=True, stop=True)
            upv = u_ps.rearrange("p hm (pr ii) -> p hm pr ii", ii=2)
            for hm in range(4):
                h = hh * 4 + hm
                for ii in range(2):
                    nc.vector.tensor_copy(
                        out=u_blk[ds(ii * DE, DE), :, ii * 8 + h],
                        in_=upv[:, hm, :, ii],
                    )

        import os as _os
        _BI = int(_os.environ.get("BISECT", "0"))

        def _emit_out(src_ap):
            t_dbg = singles.tile([P, D], f32, tag="dbg", name="dbg_out")
            nc.vector.memset(t_dbg, 0.0)
            pp = src_ap.partition_size()
            dims = list(src_ap.shape[1:])
            nfree = 1
            for s in dims:
                nfree *= s
            dst = t_dbg[ds(0, pp), ds(0, nfree)]
            if len(dims) == 2:
                dst = dst.rearrange("p (a b) -> p a b", a=dims[0])
            elif len(dims) == 3:
                dst = dst.rearrange("p (a b c) -> p a b c", a=dims[0], b=dims[1])
            nc.vector.tensor_copy(out=dst, in_=src_ap)
            nc.sync.dma_start(out=out[:, :], in_=t_dbg)

        if _BI == 1:
            _emit_out(qk_sb[:, 0, 0:2, :])
            proj_ctx.close()
            return nc
        if _BI == 8:
            ub32 = singles.tile([P, 16, 16], f32)
            nc.vector.tensor_copy(out=ub32, in_=u_blk[:, 0:16, :])
            _emit_out(ub32)
            proj_ctx.close()
            return nc
        if _BI == 7:
            _emit_out(V_sb[:, 0, 0:7, :])
            proj_ctx.close()
            return nc

        proj_ctx.close()
        stream_ctx = ExitStack()
        qe_pool = stream_ctx.enter_context(
            tc.tile_pool(name="qe", bufs=2, space="PSUM")
        )

        # ---------------- edge stream ----------------
        OV = ov_pool.tile([P, H * 33], f32)  # [128 i, (h, 33)]
        agg_ps = [
            agg_pool.tile([P, 32, 16], f32, tag=f"agg{t}", name=f"agg{t}")
            for t in range(2)
        ]
        al_t = [
            singles.tile([P, P, H], bf16, tag=f"al_{cj}", name=f"al_{cj}")
            for cj in range(2)
        ]

        # Tile may reorder independent PE ops, so cross-cj accumulations
        # cannot rely on a start=True first writer arriving first: zero the
        # accumulator regions with explicit rank-1 matmuls (order-safe).
        zrow = singles.tile([1, 512], bf16)
        nc.vector.memset(zrow, 0.0)

        def zero_mm(out_ap, m, n):
            mm(out_ap, zrow[:, :m], zrow[:, :n], start=True, stop=False)

        for t in range(2):
            zero_mm(agg_ps[t].rearrange("p a b -> p (a b)"), P, 512)
        zero_mm(OV, P, H * 33)

        def emit_agg(iq, cj):
            al = al_t[cj]
            half = iq // 2
            for prl in range(16):
                pr = iq * 16 + prl
                mm(agg_ps[half][:, pr - half * 32, :],
                   en_sb[:, cj, ds(pr * 128, 128)],
                   al[:, ds(pr * 2, 2), :],
                   start=False, stop=(cj == 1))

        def emit_outv(ihalf, cj):
            al = al_t[cj]
            for h in range(H):
                mm(OV[ds(ihalf * 64, 64), ds(h * 33, 33)],
                   al[:, ds(ihalf * 64, 64), h], V_sb[:, cj, h, :],
                   start=False, stop=False)

        for cj in range(2):
            for iq in range(4):
                qe = qe_pool.tile([P, 32, H], f32)
                for prl in range(16):
                    pr = iq * 16 + prl
                    mm(qe[:, ds(prl * 2, 2), :],
                       eT2_sb[:, cj, ds(pr * 128, 128)],
                       u_blk[:, pr, :],
                       start=True, stop=True)
                s_sum = fin_pool.tile([P, 32, H], f32, tag="s_sum")
                nc.vector.tensor_tensor(
                    out=s_sum, in0=qe,
                    in1=qk_sb[:, cj, :, ds(iq * 32, 32)].rearrange(
                        "p h i -> p i h"),
                    op=ADD,
                )
                if _BI == 2 and cj == 0 and iq == 0:
                    _emit_out(qe)
                    stream_ctx.close()
                    return nc
                nc.scalar.activation(out=al_t[cj][:, ds(iq * 32, 32), :],
                                     in_=s_sum, func=AF.Exp, scale=0.0625)
                if _BI == 3 and cj == 0 and iq == 0:
                    _emit_out(al_t[0][:, 0:32, :])
                    stream_ctx.close()
                    return nc
                # software-pipeline: agg of the previous iq
                if iq >= 1:
                    emit_agg(iq - 1, cj)
            emit_agg(3, cj)
            emit_outv(0, cj)
            emit_outv(1, cj)

        # agg diagonal blocks -> agg_sb [64 de, 128 i, 8 h] bf16
        agg_sb = singles.tile([DE, P, H], bf16)
        av = agg_sb.rearrange("p (pr ii) h -> p pr ii h", ii=2)
        for half in range(2):
            for ii in range(2):
                nc.vector.tensor_copy(
                    out=av[:, ds(half * 32, 32), ii, :],
                    in_=agg_ps[half][ds(ii * DE, DE), :, ds(ii * 8, 8)],
                )

        if _BI == 4:
            _emit_out(agg_sb[:, 0:32, :])
            stream_ctx.close()
            return nc
        if _BI == 5:
            _emit_out(OV[:, ds(0, 256)])
            stream_ctx.close()
            return nc
        # out_e: OV[i, h, :32] += agg[i, h, :] @ We_h
        for h in range(H):
            for ihalf in range(2):
                mm(OV[ds(ihalf * 64, 64), ds(h * 33, 32)],
                   agg_sb[:, ds(ihalf * 64, 64), h],
                   pk[ds(0, DE), ds(OFF_WE + h * 32, 32)],
                   start=False, stop=(h == H - 1 and ihalf == 1))

        if _BI == 6:
            _emit_out(OV[:, ds(0, 256)])
            stream_ctx.close()
            return nc

        stream_ctx.close()

        # ---------------- epilogue (all i-partitioned, no transposes) ----
        ovv = OV.rearrange("p (h c) -> p h c", c=33)
        den = singles.tile([P, H], f32)
        nc.vector.tensor_scalar_add(out=den, in0=ovv[:, :, 32], scalar1=1e-30)
        nc.vector.reciprocal(out=den, in_=den)
        outp = singles.tile([P, D], f32)
        opv = outp.rearrange("p (h c) -> p h c", c=32)
        nc.vector.tensor_tensor(
            out=opv, in0=ovv[:, :, 0:32],
            in1=den.unsqueeze(2).broadcast_to([P, H, 32]), op=MUL,
        )
        nc.vector.tensor_tensor(out=outp, in0=outp, in1=F_ps, op=ADD)
        nc.sync.dma_start(out=out[:, :], in_=outp)

    return nc


def _split_multi_waits(nc):
    """Walrus TRN2 codegen encodes at most ONE sync wait per engine
    instruction; Tile's wait assignment is not transitively minimal and
    emits 2-3.  Hoist all but one wait onto same-engine no-ops."""
    import concourse.mybir as mybir

    for fn in nc.m.functions:
        for blk in fn.blocks:
            new_insts = []
            for inst in blk.instructions:
                si = inst.sync_info
                if (
                    si is not None
                    and len(si.on_wait) > 1
                    and type(inst).__name__ != "InstEventSemaphore"
                ):
                    waits = list(si.on_wait)
                    for k, w in enumerate(waits[:-1]):
                        nop = mybir.InstNoOp(name=f"{inst.name}-sw{k}", ins=[], outs=[])
                        nop.engine = inst.engine
                        nop.sync_info = mybir.SyncInfo(on_wait=[w], on_update=[])
                        nc.register_instruction(nop)
                        new_insts.append(nop)
                    inst.sync_info = mybir.SyncInfo(
                        on_wait=[waits[-1]], on_update=list(si.on_update)
                    )
                new_insts.append(inst)
            blk.instructions = new_insts


def _get_program():
    global _PROGRAM
    if _PROGRAM is None:
        nc = _build_program()
        _split_multi_waits(nc)
        _PROGRAM = nc
    return _PROGRAM


def _prep_weights(W_q, b_q, W_k, b_k, W_v, b_v, W_e, W_s, b_s):
    """Shared (per-run) weight block of the packed buffer, bf16."""
    bf = ml_dtypes.bfloat16
    scale = np.float32(1.0 / np.sqrt(C))
    s16 = np.float32(16.0)

    def w2(w):  # (256,256) -> [128, 512] (kc-major row chunks)
        w = np.asarray(w, np.float32)
        return np.concatenate([w[0:128, :], w[128:256, :]], axis=1)

    wq = w2(np.asarray(W_q, np.float32) * s16)
    wk = w2(np.asarray(W_k, np.float32) * scale)
    wv = w2(W_v)
    ws = w2(W_s)
    wets = np.asarray(W_e, np.float32).T * scale  # [256 d', 64]
    # [32 c, 8 h, 64 de] -> rows 0-31 of a [128, 512] slot
    wets2 = np.zeros((P, 512), np.float32)
    wets2[0:32, :] = wets.reshape(H, 32, DE).transpose(1, 0, 2).reshape(32, 512)
    we = np.zeros((P, 256), np.float32)
    we[0:DE, :] = np.asarray(W_e, np.float32)
    biases = np.zeros((P, 1024 + 512), np.float32)
    biases[0, 0:256] = np.asarray(b_q, np.float32) * s16
    biases[0, 256:512] = np.asarray(b_k, np.float32) * scale
    biases[0, 512:768] = np.asarray(b_v, np.float32)
    biases[0, 768:1024] = np.asarray(b_s, np.float32)
    biases[0, 1024:1536] = 1.0
    blk = np.concatenate([wq, wk, wv, ws], axis=1)  # [128, 2048]
    return blk.astype(bf), wets2.astype(bf), we.astype(bf), biases.astype(bf)


def _prep_core_inputs(c, x, edge_attr, attn_mask, wblk):
    bf = ml_dtypes.bfloat16
    f8 = ml_dtypes.float8_e4m3
    weights, wets2, we, biases = wblk
    b, ih = c // 2, c % 2
    i0 = ih * P

    xb = np.asarray(x[b], np.float32)
    xT = np.concatenate([xb.T[0:128, :], xb.T[128:256, :]], axis=1)  # [128,512]
    xi = xb[i0 : i0 + P].T
    xTi = np.concatenate([xi[0:128, :], xi[128:256, :]], axis=1)  # [128,256]
    mb = (np.asarray(attn_mask[b, i0 : i0 + P]).T.astype(np.float32) * 800.0
          - 800.0)  # [256 j, 128 i]
    mb2 = np.concatenate([mb[0:128, :], mb[128:256, :]], axis=1)  # [128, 256]

    packed = np.empty((P, PACK_COLS), bf)
    packed[:, 0:2048] = weights
    packed[:, OFF_XT : OFF_XT + 512] = xT.astype(bf)
    packed[:, OFF_XTI : OFF_XTI + 256] = xTi.astype(bf)
    packed[:, OFF_WETS : OFF_WETS + 512] = wets2
    packed[:, OFF_WE : OFF_WE + 256] = we
    packed[:, OFF_MB : OFF_MB + 256] = mb2.astype(bf)
    packed[:, OFF_BIAS:] = biases

    ec = np.asarray(edge_attr[b, i0 : i0 + P], np.float32)  # [128 i, 256 j, 64]
    en_h = (ec.transpose(1, 0, 2).reshape(2, P, P * DE)).astype(bf)
    eT2_h = np.ascontiguousarray(
        ec.reshape(64, 2, 2, 128, DE).transpose(2, 1, 4, 0, 3)
    ).reshape(2, P, 64 * P).astype(f8)
    return {
        "packed": packed,
        "en": np.ascontiguousarray(en_h),
        "eT2": np.ascontiguousarray(eT2_h),
    }


def kernel(x, edge_attr, attn_mask, W_q, b_q, W_k, b_k, W_v, b_v, W_e, W_s, b_s):
    from concourse.bass_utils import run_bass_kernel_spmd

    x = np.asarray(x, dtype=np.float32)
    edge_attr = np.asarray(edge_attr, dtype=np.float32)
    attn_mask = np.asarray(attn_mask)
    wblk = _prep_weights(W_q, b_q, W_k, b_k, W_v, b_v, W_e, W_s, b_s)

    nc = _get_program()
    in_maps = [
        _prep_core_inputs(c, x, edge_attr, attn_mask, wblk) for c in range(NCORES)
    ]
    res = run_bass_kernel_spmd(nc, in_maps, core_ids=list(range(NCORES)))
    outv = np.empty((B, N, D), dtype=np.float32)
    for c in range(NCORES):
        b, ih = c // 2, c % 2
        outv[b, ih * P : (ih + 1) * P] = np.asarray(res.results[c]["out"])
    return outv


# revision 26
# speedup vs baseline: 2.0863x; 1.1239x over previous
"""DenseTransformerConv (GNN message passing) fused Bass/Tile kernel for Trainium2.

Sharding: 8 cores = 4 batches x 2 i-halves (data parallel; weights replicated).
Per core: b = core//2, destination-node block i in [128*(core%2), +128).

v2 design (vs v1 baseline at ~110us):
  - Edge tensor arrives from the HOST in both layouts the PE needs:
      en  [cj][128 j, 128 i, 64 de]  bf16  (j-partitioned: agg / out_v)
      eT2 [cj][128 (ii,de), 64 pr, 128 j] fp8e4m3 (de-partitioned: scores)
    -> no on-chip PE transposes, fully contiguous >=1MiB HWDGE DMAs.
  - All small tensors (weights/x/mask-bias/biases/ones) packed into ONE
    [128, 4992] bf16 buffer -> a single DMA instead of ~20.
  - No zero-fill matmuls: every PSUM accumulation group opens with start=True.
  - Scores are j-partitioned (qk batched over i); the edge-score matmul
    uses the pair-transposed fp8 tiles as 128-col stationaries (FWL-able).
  - agg is pair-batched: lhsT = en[j, (2i,64de)] (128-col stationary),
    rhs = alpha[j, (2i,8h)]; the two off-diagonal blocks are junk and the
    diagonal is extracted with 4 strided DVE copies.
  - out_v/out_e accumulate I-PARTITIONED [128 i, 8h*33]: col 33 of each head
    is a ones-column of V, so the softmax denominator falls out of the same
    matmul; normalize+skip-add are two [128,256] DVE ops. No epilogue
    transposes, no 1024-element reciprocal.
  - Scores are scaled x16 on the host (Wq,bq) so u stays in fp8 range;
    exp() applies scale=1/16.
"""

import sys

for _p in ("/opt/trn_rl_repo",):
    if _p not in sys.path:
        sys.path.append(_p)

import numpy as np
import ml_dtypes

B, N, D, DE, H, C = 4, 256, 256, 64, 8, 32
P = 128
NCORES = 8

# packed buffer column offsets (bf16 elements)
OFF_WQ, OFF_WK, OFF_WV, OFF_WS = 0, 512, 1024, 1536
OFF_XT, OFF_XTI, OFF_WETS, OFF_WE = 2048, 2560, 2816, 3328
OFF_MB, OFF_BIAS, OFF_ONES = 3584, 3840, 4864
PACK_COLS = 5376

_PROGRAM = {}


def _build_program(has_bias):
    import concourse.bass as bass
    import concourse.mybir as mybir
    import concourse.tile as tile
    from concourse.bass import ds
    from contextlib import ExitStack

    f32 = mybir.dt.float32
    bf16 = mybir.dt.bfloat16
    fp8 = mybir.dt.float8e4
    AF = mybir.ActivationFunctionType
    MUL = mybir.AluOpType.mult
    ADD = mybir.AluOpType.add

    nc = bass.Bass()

    packed = nc.declare_dram_parameter("packed", [P, PACK_COLS], bf16, isOutput=False)
    en = nc.declare_dram_parameter("en", [2, P, P * DE], bf16, isOutput=False)
    eT2 = nc.declare_dram_parameter("eT2", [2, P, 64 * P], fp8, isOutput=False)
    out = nc.declare_dram_parameter("out", [P, D], f32, isOutput=True)

    with tile.TileContext(nc) as tc, ExitStack() as ctx:
        singles = ctx.enter_context(tc.tile_pool(name="singles", bufs=1))
        fin_pool = ctx.enter_context(tc.tile_pool(name="fin", bufs=2))
        # persistent PSUM: F (skip) 1 bank, OV_i 1 bank, agg 2 banks
        f_pool = ctx.enter_context(tc.tile_pool(name="f", bufs=1, space="PSUM"))
        ov_pool = ctx.enter_context(tc.tile_pool(name="ov", bufs=1, space="PSUM"))
        agg_pool = ctx.enter_context(tc.tile_pool(name="agg", bufs=1, space="PSUM"))
        proj_ctx = ExitStack()
        proj_ps = proj_ctx.enter_context(
            tc.tile_pool(name="proj_ps", bufs=2, space="PSUM")
        )

        def mm(out_ap, lhsT, rhs, **kw):
            # every PSUM region's first writer uses start=True; order of the
            # independent regions is irrelevant -> skip sim group tracking
            kw.setdefault("skip_group_check", True)
            nc.tensor.matmul(out_ap, lhsT, rhs, **kw)

        # ---------------- one DMA for everything small ----------------
        # pk on the gpsimd (SWDGE) ring so the two HWDGE rings (sync,
        # scalar) start streaming the big edge tensors immediately.
        pk = singles.tile([P, PACK_COLS], bf16)
        nc.gpsimd.dma_start(out=pk, in_=packed[:, :])

        def w_ap(base, kc, lo, n):  # weight chunk [128, n] cols lo..lo+n
            return pk[:, ds(base + kc * 256 + lo, n)]

        ones_row = pk[ds(0, 1), ds(OFF_ONES, 512)]

        def b_row(idx, lo, n):  # bias row [1, n]
            return pk[ds(0, 1), ds(OFF_BIAS + idx * 256 + lo, n)]

        # big edge DMAs: eT2 (scores) on the sync HWDGE ring, en (agg) on
        # the scalar HWDGE ring — two rings run concurrently. Halved for
        # earlier compute start.
        eT2_sb = singles.tile([P, 2, 64 * P], fp8)
        en_sb = singles.tile([P, 2, P * DE], bf16)
        for cj in range(2):
            for hf in range(2):
                sl = ds(hf * 4096, 4096)
                nc.sync.dma_start(out=eT2_sb[:, cj, sl], in_=eT2[cj][:, sl])
                nc.scalar.dma_start(out=en_sb[:, cj, sl], in_=en[cj][:, sl])

        # ---------------- projections ----------------
        # head-split c-partitioned (PE base-partition must be 0/32/64):
        # QTi [32 c, 8 h, 128 i], KT [32 c, 8 h, 256 j] (pre-scaled)
        QTi = singles.tile([32, H, P], bf16)
        KT = singles.tile([32, H, N], bf16)
        q_ps = proj_ps.tile([32, H, P], f32, tag="proj")
        for h in range(H):
            for kc in range(2):
                mm(q_ps[:, h, :], w_ap(OFF_WQ, kc, h * 32, 32),
                   pk[:, ds(OFF_XTI + kc * 128, 128)],
                   start=(kc == 0), stop=(kc == 1 and not has_bias))
            if has_bias:
                mm(q_ps[:, h, :], b_row(0, h * 32, 32), ones_row[:, :P],
                   start=False, stop=True)
        nc.scalar.activation(out=QTi, in_=q_ps, func=AF.Copy)
        for hh in range(2):
            k_ps = proj_ps.tile([32, 4, N], f32, tag="proj")
            for hm in range(4):
                h = hh * 4 + hm
                for kc in range(2):
                    mm(k_ps[:, hm, :], w_ap(OFF_WK, kc, h * 32, 32),
                       w_ap(OFF_XT, kc, 0, 256),
                       start=(kc == 0), stop=(kc == 1 and not has_bias))
                if has_bias:
                    mm(k_ps[:, hm, :], b_row(1, h * 32, 32), ones_row[:, :N],
                       start=False, stop=True)
            nc.scalar.activation(out=KT[:, ds(hh * 4, 4), :], in_=k_ps,
                                 func=AF.Copy)

        # V [128 j, cj, 8 h, 33]: col 32 per head = 1.0 (denominator column)
        V_sb = singles.tile([P, 2, H, 33], bf16)
        nc.vector.memset(V_sb, 1.0)  # sets the ones-columns; rest overwritten
        for cj in range(2):
            v_ps = proj_ps.tile([P, D], f32, tag="proj")
            for kc in range(2):
                mm(v_ps, w_ap(OFF_XT, kc, cj * 128, 128),
                   w_ap(OFF_WV, kc, 0, 256),
                   start=(kc == 0), stop=(kc == 1 and not has_bias))
            if has_bias:
                mm(v_ps, ones_row[:, :P], b_row(2, 0, 256),
                   start=False, stop=True)
            nc.vector.tensor_copy(
                out=V_sb[:, cj, :, 0:32],
                in_=v_ps.rearrange("p (h c) -> p h c", h=H),
            )

        # skip connection F = xTi^T @ Ws + bs  (i-partitioned, kept open)
        F_ps = f_pool.tile([P, D], f32)
        for kc in range(2):
            mm(F_ps, pk[:, ds(OFF_XTI + kc * 128, 128)], w_ap(OFF_WS, kc, 0, 256),
               start=(kc == 0), stop=(kc == 1 and not has_bias))
        if has_bias:
            mm(F_ps, ones_row[:, :P], b_row(3, 0, 256), start=False, stop=True)

        # QK scores + mask -> qk_sb [128 j, cj, 8 h, 128 i] bf16 (x16 scaled)
        qk_sb = singles.tile([P, 2, H, P], bf16)
        for cj in range(2):
            qk_ps = proj_ps.tile([P, H, P], f32, tag="proj")
            for h in range(H):
                mm(qk_ps[:, h, :], KT[:, h, ds(cj * 128, 128)],
                   QTi[:, h, :], start=True, stop=True)
            nc.vector.tensor_tensor(
                out=qk_sb[:, cj, :, :],
                in0=qk_ps,
                in1=pk[:, ds(OFF_MB + cj * 128, 128)]
                .unsqueeze(1).broadcast_to([P, H, P]),
                op=ADD,
            )

        # u_blk: block-diag fp8 [128 (ii,de), 64 pr, 16 (ii,h)]
        u_blk = singles.tile([P, 64, 16], fp8)
        nc.gpsimd.memset(u_blk, 0.0)
        for hh in range(2):
            u_ps = proj_ps.tile([DE, 4, P], f32, tag="proj")
            for hm in range(4):
                h = hh * 4 + hm
                mm(u_ps[:, hm, :], pk[ds(0, 32), ds(OFF_WETS + h * 64, 64)],
                   QTi[:, h, :], start=True, stop=True)
            upv = u_ps.rearrange("p hm (pr ii) -> p pr ii hm", ii=2)
            for ii in range(2):
                dst = u_blk[ds(ii * DE, DE), :, ds(ii * 8 + hh * 4, 4)]
                if ii == 0:
                    nc.vector.tensor_copy(out=dst, in_=upv[:, :, ii, :])
                else:
                    nc.scalar.activation(out=dst, in_=upv[:, :, ii, :],
                                         func=AF.Copy)

        import os as _os
        _BI = int(_os.environ.get("BISECT", "0"))

        def _emit_out(src_ap):
            t_dbg = singles.tile([P, D], f32, tag="dbg", name="dbg_out")
            nc.vector.memset(t_dbg, 0.0)
            pp = src_ap.partition_size()
            dims = list(src_ap.shape[1:])
            nfree = 1
            for s in dims:
                nfree *= s
            dst = t_dbg[ds(0, pp), ds(0, nfree)]
            if len(dims) == 2:
                dst = dst.rearrange("p (a b) -> p a b", a=dims[0])
            elif len(dims) == 3:
                dst = dst.rearrange("p (a b c) -> p a b c", a=dims[0], b=dims[1])
            nc.vector.tensor_copy(out=dst, in_=src_ap)
            nc.sync.dma_start(out=out[:, :], in_=t_dbg)

        if _BI == 1:
            _emit_out(qk_sb[:, 0, 0:2, :])
            proj_ctx.close()
            return nc
        if _BI == 8:
            ub32 = singles.tile([P, 16, 16], f32)
            nc.vector.tensor_copy(out=ub32, in_=u_blk[:, 0:16, :])
            _emit_out(ub32)
            proj_ctx.close()
            return nc
        if _BI == 7:
            _emit_out(V_sb[:, 0, 0:7, :])
            proj_ctx.close()
            return nc

        proj_ctx.close()
        stream_ctx = ExitStack()
        qe_pool = stream_ctx.enter_context(
            tc.tile_pool(name="qe", bufs=2, space="PSUM")
        )

        # ---------------- edge stream ----------------
        OV = ov_pool.tile([P, H * 33], f32)  # [128 i, (h, 33)]
        agg_ps = [
            agg_pool.tile([P, 32, 16], f32, tag=f"agg{t}", name=f"agg{t}")
            for t in range(2)
        ]
        al_t = [
            singles.tile([P, H, P], bf16, tag=f"al_{cj}", name=f"al_{cj}")
            for cj in range(2)
        ]

        # Tile may reorder independent PE ops, so cross-cj accumulations
        # cannot rely on a start=True first writer arriving first: zero the
        # accumulator regions with explicit rank-1 matmuls (order-safe).
        zrow = singles.tile([1, 512], bf16)
        nc.vector.memset(zrow, 0.0)

        def zero_mm(out_ap, m, n):
            mm(out_ap, zrow[:, :m], zrow[:, :n], start=True, stop=False)

        for t in range(2):
            zero_mm(agg_ps[t].rearrange("p a b -> p (a b)"), P, 512)
        zero_mm(OV, P, H * 33)

        def emit_agg(iq, cj):
            al = al_t[cj]
            half = iq // 2
            for prl in range(16):
                pr = iq * 16 + prl
                mm(agg_ps[half][:, pr - half * 32, :],
                   en_sb[:, cj, ds(pr * 128, 128)],
                   al.rearrange("p h i -> p i h")[:, ds(pr * 2, 2), :],
                   start=False, stop=(cj == 1))

        def emit_outv(ihalf, cj):
            al = al_t[cj]
            for h in range(H):
                mm(OV[ds(ihalf * 64, 64), ds(h * 33, 33)],
                   al[:, h, ds(ihalf * 64, 64)], V_sb[:, cj, h, :],
                   start=False, stop=False)

        for cj in range(2):
            for iq in range(4):
                qe = qe_pool.tile([P, H, 32], f32)
                qe_v = qe.rearrange("p h i -> p i h")
                for prl in range(16):
                    pr = iq * 16 + prl
                    mm(qe_v[:, ds(prl * 2, 2), :],
                       eT2_sb[:, cj, ds(pr * 128, 128)],
                       u_blk[:, pr, :],
                       start=True, stop=True)
                s_sum = fin_pool.tile([P, H, 32], f32, tag="s_sum")
                nc.vector.tensor_tensor(
                    out=s_sum, in0=qe,
                    in1=qk_sb[:, cj, :, ds(iq * 32, 32)],
                    op=ADD,
                )
                if _BI == 2 and cj == 0 and iq == 0:
                    _emit_out(qe)
                    stream_ctx.close()
                    return nc
                nc.scalar.activation(out=al_t[cj][:, :, ds(iq * 32, 32)],
                                     in_=s_sum, func=AF.Exp, scale=0.0625)
                if _BI == 3 and cj == 0 and iq == 0:
                    _emit_out(al_t[0][:, :, 0:32])
                    stream_ctx.close()
                    return nc
                # software-pipeline: agg of the previous iq
                if iq >= 1:
                    emit_agg(iq - 1, cj)
            emit_agg(3, cj)
            emit_outv(0, cj)
            emit_outv(1, cj)

        # agg diagonal blocks -> agg_sb [64 de, 128 i, 8 h] bf16
        agg_sb = singles.tile([DE, P, H], bf16)
        av = agg_sb.rearrange("p (pr ii) h -> p pr ii h", ii=2)
        for half in range(2):
            for ii in range(2):
                nc.vector.tensor_copy(
                    out=av[:, ds(half * 32, 32), ii, :],
                    in_=agg_ps[half][ds(ii * DE, DE), :, ds(ii * 8, 8)],
                )

        if _BI == 4:
            _emit_out(agg_sb[:, 0:32, :])
            stream_ctx.close()
            return nc
        if _BI == 5:
            _emit_out(OV[:, ds(0, 256)])
            stream_ctx.close()
            return nc
        # out_e: OV[i, h, :32] += agg[i, h, :] @ We_h
        for h in range(H):
            for ihalf in range(2):
                mm(OV[ds(ihalf * 64, 64), ds(h * 33, 32)],
                   agg_sb[:, ds(ihalf * 64, 64), h],
                   pk[ds(0, DE), ds(OFF_WE + h * 32, 32)],
                   start=False, stop=(h == H - 1 and ihalf == 1))

        if _BI == 6:
            _emit_out(OV[:, ds(0, 256)])
            stream_ctx.close()
            return nc

        stream_ctx.close()

        # ---------------- epilogue (all i-partitioned, no transposes) ----
        ovv = OV.rearrange("p (h c) -> p h c", c=33)
        den = singles.tile([P, H], f32)
        nc.vector.tensor_scalar_add(out=den, in0=ovv[:, :, 32], scalar1=1e-30)
        nc.vector.reciprocal(out=den, in_=den)
        outp = singles.tile([P, D], f32)
        opv = outp.rearrange("p (h c) -> p h c", c=32)
        nc.vector.tensor_tensor(
            out=opv, in0=ovv[:, :, 0:32],
            in1=den.unsqueeze(2).broadcast_to([P, H, 32]), op=MUL,
        )
        nc.vector.tensor_tensor(out=outp, in0=outp, in1=F_ps, op=ADD)
        nc.sync.dma_start(out=out[:, :], in_=outp)

    return nc


def _split_multi_waits(nc):
    """Walrus TRN2 codegen encodes at most ONE sync wait per engine
    instruction; Tile's wait assignment is not transitively minimal and
    emits 2-3.  Hoist all but one wait onto same-engine no-ops."""
    import concourse.mybir as mybir

    for fn in nc.m.functions:
        for blk in fn.blocks:
            new_insts = []
            for inst in blk.instructions:
                si = inst.sync_info
                if (
                    si is not None
                    and len(si.on_wait) > 1
                    and type(inst).__name__ != "InstEventSemaphore"
                ):
                    waits = list(si.on_wait)
                    for k, w in enumerate(waits[:-1]):
                        nop = mybir.InstNoOp(name=f"{inst.name}-sw{k}", ins=[], outs=[])
                        nop.engine = inst.engine
                        nop.sync_info = mybir.SyncInfo(on_wait=[w], on_update=[])
                        nc.register_instruction(nop)
                        new_insts.append(nop)
                    inst.sync_info = mybir.SyncInfo(
                        on_wait=[waits[-1]], on_update=list(si.on_update)
                    )
                new_insts.append(inst)
            blk.instructions = new_insts


def _get_program(has_bias=False):
    if has_bias not in _PROGRAM:
        nc = _build_program(has_bias)
        _split_multi_waits(nc)
        _PROGRAM[has_bias] = nc
    return _PROGRAM[has_bias]


def _prep_weights(W_q, b_q, W_k, b_k, W_v, b_v, W_e, W_s, b_s):
    """Shared (per-run) weight block of the packed buffer, bf16."""
    bf = ml_dtypes.bfloat16
    scale = np.float32(1.0 / np.sqrt(C))
    s16 = np.float32(16.0)

    def w2(w):  # (256,256) -> [128, 512] (kc-major row chunks)
        w = np.asarray(w, np.float32)
        return np.concatenate([w[0:128, :], w[128:256, :]], axis=1)

    wq = w2(np.asarray(W_q, np.float32) * s16)
    wk = w2(np.asarray(W_k, np.float32) * scale)
    wv = w2(W_v)
    ws = w2(W_s)
    wets = np.asarray(W_e, np.float32).T * scale  # [256 d', 64]
    # [32 c, 8 h, 64 de] -> rows 0-31 of a [128, 512] slot
    wets2 = np.zeros((P, 512), np.float32)
    wets2[0:32, :] = wets.reshape(H, 32, DE).transpose(1, 0, 2).reshape(32, 512)
    we = np.zeros((P, 256), np.float32)
    we[0:DE, :] = np.asarray(W_e, np.float32)
    biases = np.zeros((P, 1024 + 512), np.float32)
    biases[0, 0:256] = np.asarray(b_q, np.float32) * s16
    biases[0, 256:512] = np.asarray(b_k, np.float32) * scale
    biases[0, 512:768] = np.asarray(b_v, np.float32)
    biases[0, 768:1024] = np.asarray(b_s, np.float32)
    biases[0, 1024:1536] = 1.0
    blk = np.concatenate([wq, wk, wv, ws], axis=1)  # [128, 2048]
    has_bias = bool(
        np.any(np.asarray(b_q)) or np.any(np.asarray(b_k))
        or np.any(np.asarray(b_v)) or np.any(np.asarray(b_s))
    )
    return blk.astype(bf), wets2.astype(bf), we.astype(bf), biases.astype(bf), has_bias


def _prep_core_inputs(c, x, edge_attr, attn_mask, wblk):
    bf = ml_dtypes.bfloat16
    f8 = ml_dtypes.float8_e4m3
    weights, wets2, we, biases, _has_bias = wblk
    b, ih = c // 2, c % 2
    i0 = ih * P

    xb = np.asarray(x[b], np.float32)
    xT = np.concatenate([xb.T[0:128, :], xb.T[128:256, :]], axis=1)  # [128,512]
    xi = xb[i0 : i0 + P].T
    xTi = np.concatenate([xi[0:128, :], xi[128:256, :]], axis=1)  # [128,256]
    mb = (np.asarray(attn_mask[b, i0 : i0 + P]).T.astype(np.float32) * 800.0
          - 800.0)  # [256 j, 128 i]
    mb2 = np.concatenate([mb[0:128, :], mb[128:256, :]], axis=1)  # [128, 256]

    packed = np.empty((P, PACK_COLS), bf)
    packed[:, 0:2048] = weights
    packed[:, OFF_XT : OFF_XT + 512] = xT.astype(bf)
    packed[:, OFF_XTI : OFF_XTI + 256] = xTi.astype(bf)
    packed[:, OFF_WETS : OFF_WETS + 512] = wets2
    packed[:, OFF_WE : OFF_WE + 256] = we
    packed[:, OFF_MB : OFF_MB + 256] = mb2.astype(bf)
    packed[:, OFF_BIAS:] = biases

    ec = np.asarray(edge_attr[b, i0 : i0 + P], np.float32)  # [128 i, 256 j, 64]
    en_h = (ec.transpose(1, 0, 2).reshape(2, P, P * DE)).astype(bf)
    eT2_h = np.ascontiguousarray(
        ec.reshape(64, 2, 2, 128, DE).transpose(2, 1, 4, 0, 3)
    ).reshape(2, P, 64 * P).astype(f8)
    return {
        "packed": packed,
        "en": np.ascontiguousarray(en_h),
        "eT2": np.ascontiguousarray(eT2_h),
    }


def kernel(x, edge_attr, attn_mask, W_q, b_q, W_k, b_k, W_v, b_v, W_e, W_s, b_s):
    from concourse.bass_utils import run_bass_kernel_spmd

    x = np.asarray(x, dtype=np.float32)
    edge_attr = np.asarray(edge_attr, dtype=np.float32)
    attn_mask = np.asarray(attn_mask)
    wblk = _prep_weights(W_q, b_q, W_k, b_k, W_v, b_v, W_e, W_s, b_s)

    nc = _get_program(wblk[-1])
    in_maps = [
        _prep_core_inputs(c, x, edge_attr, attn_mask, wblk) for c in range(NCORES)
    ]
    res = run_bass_kernel_spmd(nc, in_maps, core_ids=list(range(NCORES)))
    outv = np.empty((B, N, D), dtype=np.float32)
    for c in range(NCORES):
        b, ih = c // 2, c % 2
        outv[b, ih * P : (ih + 1) * P] = np.asarray(res.results[c]["out"])
    return outv


# revision 27
# speedup vs baseline: 2.4235x; 1.1616x over previous
"""DenseTransformerConv (GNN message passing) fused Bass/Tile kernel for Trainium2.

Sharding: 8 cores = 4 batches x 2 i-halves (data parallel; weights replicated).
Per core: b = core//2, destination-node block i in [128*(core%2), +128).

v2 design (vs v1 baseline at ~110us):
  - Edge tensor arrives from the HOST in both layouts the PE needs:
      en  [cj][128 j, 128 i, 64 de]  bf16  (j-partitioned: agg / out_v)
      eT2 [cj][128 (ii,de), 64 pr, 128 j] fp8e4m3 (de-partitioned: scores)
    -> no on-chip PE transposes, fully contiguous >=1MiB HWDGE DMAs.
  - All small tensors (weights/x/mask-bias/biases/ones) packed into ONE
    [128, 4992] bf16 buffer -> a single DMA instead of ~20.
  - No zero-fill matmuls: every PSUM accumulation group opens with start=True.
  - Scores are j-partitioned (qk batched over i); the edge-score matmul
    uses the pair-transposed fp8 tiles as 128-col stationaries (FWL-able).
  - agg is pair-batched: lhsT = en[j, (2i,64de)] (128-col stationary),
    rhs = alpha[j, (2i,8h)]; the two off-diagonal blocks are junk and the
    diagonal is extracted with 4 strided DVE copies.
  - out_v/out_e accumulate I-PARTITIONED [128 i, 8h*33]: col 33 of each head
    is a ones-column of V, so the softmax denominator falls out of the same
    matmul; normalize+skip-add are two [128,256] DVE ops. No epilogue
    transposes, no 1024-element reciprocal.
  - Scores are scaled x16 on the host (Wq,bq) so u stays in fp8 range;
    exp() applies scale=1/16.
"""

import sys

for _p in ("/opt/trn_rl_repo",):
    if _p not in sys.path:
        sys.path.append(_p)

import numpy as np
import ml_dtypes

B, N, D, DE, H, C = 4, 256, 256, 64, 8, 32
P = 128
NCORES = 8

# packed buffer column offsets (bf16 elements)
OFF_WQ, OFF_WK, OFF_WV, OFF_WS = 0, 512, 1024, 1536
OFF_XT, OFF_XTI, OFF_WETS, OFF_WE = 2048, 2560, 2816, 3328
OFF_MB, OFF_BIAS, OFF_ONES = 3584, 3840, 4864
PACK_COLS = 5376

_PROGRAM = {}


def _build_program(has_bias):
    import concourse.bass as bass
    import concourse.mybir as mybir
    import concourse.tile as tile
    from concourse.bass import ds
    from contextlib import ExitStack

    f32 = mybir.dt.float32
    bf16 = mybir.dt.bfloat16
    fp8 = mybir.dt.float8e4
    AF = mybir.ActivationFunctionType
    MUL = mybir.AluOpType.mult
    ADD = mybir.AluOpType.add

    nc = bass.Bass()

    packed = nc.declare_dram_parameter("packed", [P, PACK_COLS], bf16, isOutput=False)
    en = nc.declare_dram_parameter("en", [2, P, P * DE], bf16, isOutput=False)
    eT2 = nc.declare_dram_parameter("eT2", [2, P, 64 * P], fp8, isOutput=False)
    out = nc.declare_dram_parameter("out", [P, D], f32, isOutput=True)

    with tile.TileContext(nc) as tc, ExitStack() as ctx:
        singles = ctx.enter_context(tc.tile_pool(name="singles", bufs=1))
        fin_pool = ctx.enter_context(tc.tile_pool(name="fin", bufs=2))
        # persistent PSUM: F (skip) 1 bank, OV_i 1 bank, agg 2 banks
        f_pool = ctx.enter_context(tc.tile_pool(name="f", bufs=1, space="PSUM"))
        ov_pool = ctx.enter_context(tc.tile_pool(name="ov", bufs=1, space="PSUM"))
        agg_pool = ctx.enter_context(tc.tile_pool(name="agg", bufs=1, space="PSUM"))
        proj_ctx = ExitStack()
        proj_ps = proj_ctx.enter_context(
            tc.tile_pool(name="proj_ps", bufs=2, space="PSUM")
        )

        def mm(out_ap, lhsT, rhs, **kw):
            # every PSUM region's first writer uses start=True; order of the
            # independent regions is irrelevant -> skip sim group tracking
            kw.setdefault("skip_group_check", True)
            nc.tensor.matmul(out_ap, lhsT, rhs, **kw)

        # ---------------- one DMA for everything small ----------------
        # pk first on the scalar HWDGE ring (prologue gates on it); the
        # sync HWDGE ring starts on eT2 concurrently.
        pk = singles.tile([P, PACK_COLS], bf16)
        nc.scalar.dma_start(out=pk, in_=packed[:, :])

        def w_ap(base, kc, lo, n):  # weight chunk [128, n] cols lo..lo+n
            return pk[:, ds(base + kc * 256 + lo, n)]

        ones_row = pk[ds(0, 1), ds(OFF_ONES, 512)]

        def b_row(idx, lo, n):  # bias row [1, n]
            return pk[ds(0, 1), ds(OFF_BIAS + idx * 256 + lo, n)]

        # big edge DMAs: eT2 (scores) on the sync HWDGE ring, en (agg) on
        # the scalar HWDGE ring — two rings run concurrently. Halved for
        # earlier compute start.
        eT2_sb = singles.tile([P, 2, 64 * P], fp8)
        en_sb = singles.tile([P, 2, P * DE], bf16)
        for cj in range(2):
            for hf in range(2):
                sl = ds(hf * 4096, 4096)
                nc.sync.dma_start(out=eT2_sb[:, cj, sl], in_=eT2[cj][:, sl])
                nc.scalar.dma_start(out=en_sb[:, cj, sl], in_=en[cj][:, sl])

        # ---------------- projections ----------------
        # head-split c-partitioned (PE base-partition must be 0/32/64):
        # QTi [32 c, 8 h, 128 i], KT [32 c, 8 h, 256 j] (pre-scaled)
        QTi = singles.tile([32, H, P], bf16)
        KT = singles.tile([32, H, N], bf16)
        q_ps = proj_ps.tile([32, H, P], f32, tag="proj")
        for h in range(H):
            for kc in range(2):
                mm(q_ps[:, h, :], w_ap(OFF_WQ, kc, h * 32, 32),
                   pk[:, ds(OFF_XTI + kc * 128, 128)],
                   start=(kc == 0), stop=(kc == 1 and not has_bias))
            if has_bias:
                mm(q_ps[:, h, :], b_row(0, h * 32, 32), ones_row[:, :P],
                   start=False, stop=True)
        nc.scalar.activation(out=QTi, in_=q_ps, func=AF.Copy)
        for hh in range(2):
            k_ps = proj_ps.tile([32, 4, N], f32, tag="proj")
            for hm in range(4):
                h = hh * 4 + hm
                for kc in range(2):
                    mm(k_ps[:, hm, :], w_ap(OFF_WK, kc, h * 32, 32),
                       w_ap(OFF_XT, kc, 0, 256),
                       start=(kc == 0), stop=(kc == 1 and not has_bias))
                if has_bias:
                    mm(k_ps[:, hm, :], b_row(1, h * 32, 32), ones_row[:, :N],
                       start=False, stop=True)
            nc.scalar.activation(out=KT[:, ds(hh * 4, 4), :], in_=k_ps,
                                 func=AF.Copy)

        # V [128 j, cj, 8 h, 33]: col 32 per head = 1.0 (denominator column)
        V_sb = singles.tile([P, 2, H, 33], bf16)
        nc.vector.memset(V_sb, 1.0)  # sets the ones-columns; rest overwritten
        for cj in range(2):
            v_ps = proj_ps.tile([P, D], f32, tag="proj")
            for kc in range(2):
                mm(v_ps, w_ap(OFF_XT, kc, cj * 128, 128),
                   w_ap(OFF_WV, kc, 0, 256),
                   start=(kc == 0), stop=(kc == 1 and not has_bias))
            if has_bias:
                mm(v_ps, ones_row[:, :P], b_row(2, 0, 256),
                   start=False, stop=True)
            nc.vector.tensor_copy(
                out=V_sb[:, cj, :, 0:32],
                in_=v_ps.rearrange("p (h c) -> p h c", h=H),
            )

        # skip connection F = xTi^T @ Ws + bs  (i-partitioned, kept open)
        F_ps = f_pool.tile([P, D], f32)
        for kc in range(2):
            mm(F_ps, pk[:, ds(OFF_XTI + kc * 128, 128)], w_ap(OFF_WS, kc, 0, 256),
               start=(kc == 0), stop=(kc == 1 and not has_bias))
        if has_bias:
            mm(F_ps, ones_row[:, :P], b_row(3, 0, 256), start=False, stop=True)

        # QK scores + mask -> qk_sb [128 j, cj, 8 h, 128 i] bf16 (x16 scaled)
        qk_sb = singles.tile([P, 2, H, P], bf16)
        for cj in range(2):
            qk_ps = proj_ps.tile([P, H, P], f32, tag="proj")
            for h in range(H):
                mm(qk_ps[:, h, :], KT[:, h, ds(cj * 128, 128)],
                   QTi[:, h, :], start=True, stop=True)
            nc.vector.tensor_tensor(
                out=qk_sb[:, cj, :, :],
                in0=qk_ps,
                in1=pk[:, ds(OFF_MB + cj * 128, 128)]
                .unsqueeze(1).broadcast_to([P, H, P]),
                op=ADD,
            )

        # u_blk: block-diag fp8 [128 (ii,de), 64 pr, 16 (ii,h)]
        u_blk = singles.tile([P, 64, 16], fp8)
        nc.gpsimd.memset(u_blk, 0.0)
        for hh in range(2):
            u_ps = proj_ps.tile([DE, 4, P], f32, tag="proj")
            for hm in range(4):
                h = hh * 4 + hm
                mm(u_ps[:, hm, :], pk[ds(0, 32), ds(OFF_WETS + h * 64, 64)],
                   QTi[:, h, :], start=True, stop=True)
            upv = u_ps.rearrange("p hm (pr ii) -> p pr ii hm", ii=2)
            for ii in range(2):
                dst = u_blk[ds(ii * DE, DE), :, ds(ii * 8 + hh * 4, 4)]
                if ii == 0:
                    nc.vector.tensor_copy(out=dst, in_=upv[:, :, ii, :])
                else:
                    nc.scalar.activation(out=dst, in_=upv[:, :, ii, :],
                                         func=AF.Copy)

        import os as _os
        _BI = int(_os.environ.get("BISECT", "0"))

        def _emit_out(src_ap):
            t_dbg = singles.tile([P, D], f32, tag="dbg", name="dbg_out")
            nc.vector.memset(t_dbg, 0.0)
            pp = src_ap.partition_size()
            dims = list(src_ap.shape[1:])
            nfree = 1
            for s in dims:
                nfree *= s
            dst = t_dbg[ds(0, pp), ds(0, nfree)]
            if len(dims) == 2:
                dst = dst.rearrange("p (a b) -> p a b", a=dims[0])
            elif len(dims) == 3:
                dst = dst.rearrange("p (a b c) -> p a b c", a=dims[0], b=dims[1])
            nc.vector.tensor_copy(out=dst, in_=src_ap)
            nc.sync.dma_start(out=out[:, :], in_=t_dbg)

        if _BI == 1:
            _emit_out(qk_sb[:, 0, 0:2, :])
            proj_ctx.close()
            return nc
        if _BI == 8:
            ub32 = singles.tile([P, 16, 16], f32)
            nc.vector.tensor_copy(out=ub32, in_=u_blk[:, 0:16, :])
            _emit_out(ub32)
            proj_ctx.close()
            return nc
        if _BI == 7:
            _emit_out(V_sb[:, 0, 0:7, :])
            proj_ctx.close()
            return nc

        proj_ctx.close()
        stream_ctx = ExitStack()
        qe_pool = stream_ctx.enter_context(
            tc.tile_pool(name="qe", bufs=2, space="PSUM")
        )

        # ---------------- edge stream ----------------
        OV = ov_pool.tile([P, H * 33], f32)  # [128 i, (h, 33)]
        agg_ps = [
            agg_pool.tile([P, 32, 16], f32, tag=f"agg{t}", name=f"agg{t}")
            for t in range(2)
        ]
        al_t = [
            singles.tile([P, H, P], bf16, tag=f"al_{cj}", name=f"al_{cj}")
            for cj in range(2)
        ]

        # Tile may reorder independent PE ops, so cross-cj accumulations
        # cannot rely on a start=True first writer arriving first: zero the
        # accumulator regions with explicit rank-1 matmuls (order-safe).
        zrow = singles.tile([1, 512], bf16)
        nc.vector.memset(zrow, 0.0)

        def zero_mm(out_ap, m, n):
            mm(out_ap, zrow[:, :m], zrow[:, :n], start=True, stop=False)

        for t in range(2):
            zero_mm(agg_ps[t].rearrange("p a b -> p (a b)"), P, 512)
        zero_mm(OV, P, H * 33)

        def emit_agg(iq, cj):
            al = al_t[cj]
            half = iq // 2
            for prl in range(16):
                pr = iq * 16 + prl
                mm(agg_ps[half][:, pr - half * 32, :],
                   en_sb[:, cj, ds(pr * 128, 128)],
                   al.rearrange("p h i -> p i h")[:, ds(pr * 2, 2), :],
                   start=False, stop=(cj == 1))

        def emit_outv(ihalf, cj):
            al = al_t[cj]
            for h in range(H):
                mm(OV[ds(ihalf * 64, 64), ds(h * 33, 33)],
                   al[:, h, ds(ihalf * 64, 64)], V_sb[:, cj, h, :],
                   start=False, stop=False)

        for cj in range(2):
            for iq in range(4):
                qe = qe_pool.tile([P, H, 32], f32)
                qe_v = qe.rearrange("p h i -> p i h")
                for prl in range(16):
                    pr = iq * 16 + prl
                    mm(qe_v[:, ds(prl * 2, 2), :],
                       eT2_sb[:, cj, ds(pr * 128, 128)],
                       u_blk[:, pr, :],
                       start=True, stop=True)
                s_sum = fin_pool.tile([P, H, 32], f32, tag="s_sum")
                nc.vector.tensor_tensor(
                    out=s_sum, in0=qe,
                    in1=qk_sb[:, cj, :, ds(iq * 32, 32)],
                    op=ADD,
                )
                if _BI == 2 and cj == 0 and iq == 0:
                    _emit_out(qe)
                    stream_ctx.close()
                    return nc
                nc.scalar.activation(out=al_t[cj][:, :, ds(iq * 32, 32)],
                                     in_=s_sum, func=AF.Exp, scale=0.0625)
                if _BI == 3 and cj == 0 and iq == 0:
                    _emit_out(al_t[0][:, :, 0:32])
                    stream_ctx.close()
                    return nc
                # software-pipeline: agg of the previous iq
                if iq >= 1:
                    emit_agg(iq - 1, cj)
            emit_agg(3, cj)
            emit_outv(0, cj)
            emit_outv(1, cj)

        # agg diagonal blocks -> agg_sb [64 de, 128 i, 8 h] bf16
        agg_sb = singles.tile([DE, P, H], bf16)
        av = agg_sb.rearrange("p (pr ii) h -> p pr ii h", ii=2)
        for half in range(2):
            for ii in range(2):
                nc.vector.tensor_copy(
                    out=av[:, ds(half * 32, 32), ii, :],
                    in_=agg_ps[half][ds(ii * DE, DE), :, ds(ii * 8, 8)],
                )

        if _BI == 4:
            _emit_out(agg_sb[:, 0:32, :])
            stream_ctx.close()
            return nc
        if _BI == 5:
            _emit_out(OV[:, ds(0, 256)])
            stream_ctx.close()
            return nc
        # out_e: OV[i, h, :32] += agg[i, h, :] @ We_h
        for h in range(H):
            for ihalf in range(2):
                mm(OV[ds(ihalf * 64, 64), ds(h * 33, 32)],
                   agg_sb[:, ds(ihalf * 64, 64), h],
                   pk[ds(0, DE), ds(OFF_WE + h * 32, 32)],
                   start=False, stop=(h == H - 1 and ihalf == 1))

        if _BI == 6:
            _emit_out(OV[:, ds(0, 256)])
            stream_ctx.close()
            return nc

        stream_ctx.close()

        # ---------------- epilogue (all i-partitioned, no transposes) ----
        ovv = OV.rearrange("p (h c) -> p h c", c=33)
        den = singles.tile([P, H], f32)
        nc.vector.tensor_scalar_add(out=den, in0=ovv[:, :, 32], scalar1=1e-30)
        nc.vector.reciprocal(out=den, in_=den)
        outp = singles.tile([P, D], f32)
        opv = outp.rearrange("p (h c) -> p h c", c=32)
        nc.vector.tensor_tensor(
            out=opv, in0=ovv[:, :, 0:32],
            in1=den.unsqueeze(2).broadcast_to([P, H, 32]), op=MUL,
        )
        nc.vector.tensor_tensor(out=outp, in0=outp, in1=F_ps, op=ADD)
        nc.sync.dma_start(out=out[:, :], in_=outp)

    return nc


def _split_multi_waits(nc):
    """Walrus TRN2 codegen encodes at most ONE sync wait per engine
    instruction; Tile's wait assignment is not transitively minimal and
    emits 2-3.  Hoist all but one wait onto same-engine no-ops."""
    import concourse.mybir as mybir

    for fn in nc.m.functions:
        for blk in fn.blocks:
            new_insts = []
            for inst in blk.instructions:
                si = inst.sync_info
                if (
                    si is not None
                    and len(si.on_wait) > 1
                    and type(inst).__name__ != "InstEventSemaphore"
                ):
                    waits = list(si.on_wait)
                    for k, w in enumerate(waits[:-1]):
                        nop = mybir.InstNoOp(name=f"{inst.name}-sw{k}", ins=[], outs=[])
                        nop.engine = inst.engine
                        nop.sync_info = mybir.SyncInfo(on_wait=[w], on_update=[])
                        nc.register_instruction(nop)
                        new_insts.append(nop)
                    inst.sync_info = mybir.SyncInfo(
                        on_wait=[waits[-1]], on_update=list(si.on_update)
                    )
                new_insts.append(inst)
            blk.instructions = new_insts


def _get_program(has_bias=False):
    if has_bias not in _PROGRAM:
        nc = _build_program(has_bias)
        _split_multi_waits(nc)
        _PROGRAM[has_bias] = nc
    return _PROGRAM[has_bias]


def _prep_weights(W_q, b_q, W_k, b_k, W_v, b_v, W_e, W_s, b_s):
    """Shared (per-run) weight block of the packed buffer, bf16."""
    bf = ml_dtypes.bfloat16
    scale = np.float32(1.0 / np.sqrt(C))
    s16 = np.float32(16.0)

    def w2(w):  # (256,256) -> [128, 512] (kc-major row chunks)
        w = np.asarray(w, np.float32)
        return np.concatenate([w[0:128, :], w[128:256, :]], axis=1)

    wq = w2(np.asarray(W_q, np.float32) * s16)
    wk = w2(np.asarray(W_k, np.float32) * scale)
    wv = w2(W_v)
    ws = w2(W_s)
    wets = np.asarray(W_e, np.float32).T * scale  # [256 d', 64]
    # [32 c, 8 h, 64 de] -> rows 0-31 of a [128, 512] slot
    wets2 = np.zeros((P, 512), np.float32)
    wets2[0:32, :] = wets.reshape(H, 32, DE).transpose(1, 0, 2).reshape(32, 512)
    we = np.zeros((P, 256), np.float32)
    we[0:DE, :] = np.asarray(W_e, np.float32)
    biases = np.zeros((P, 1024 + 512), np.float32)
    biases[0, 0:256] = np.asarray(b_q, np.float32) * s16
    biases[0, 256:512] = np.asarray(b_k, np.float32) * scale
    biases[0, 512:768] = np.asarray(b_v, np.float32)
    biases[0, 768:1024] = np.asarray(b_s, np.float32)
    biases[0, 1024:1536] = 1.0
    blk = np.concatenate([wq, wk, wv, ws], axis=1)  # [128, 2048]
    has_bias = bool(
        np.any(np.asarray(b_q)) or np.any(np.asarray(b_k))
        or np.any(np.asarray(b_v)) or np.any(np.asarray(b_s))
    )
    return blk.astype(bf), wets2.astype(bf), we.astype(bf), biases.astype(bf), has_bias


def _prep_core_inputs(c, x, edge_attr, attn_mask, wblk):
    bf = ml_dtypes.bfloat16
    f8 = ml_dtypes.float8_e4m3
    weights, wets2, we, biases, _has_bias = wblk
    b, ih = c // 2, c % 2
    i0 = ih * P

    xb = np.asarray(x[b], np.float32)
    xT = np.concatenate([xb.T[0:128, :], xb.T[128:256, :]], axis=1)  # [128,512]
    xi = xb[i0 : i0 + P].T
    xTi = np.concatenate([xi[0:128, :], xi[128:256, :]], axis=1)  # [128,256]
    mb = (np.asarray(attn_mask[b, i0 : i0 + P]).T.astype(np.float32) * 800.0
          - 800.0)  # [256 j, 128 i]
    mb2 = np.concatenate([mb[0:128, :], mb[128:256, :]], axis=1)  # [128, 256]

    packed = np.empty((P, PACK_COLS), bf)
    packed[:, 0:2048] = weights
    packed[:, OFF_XT : OFF_XT + 512] = xT.astype(bf)
    packed[:, OFF_XTI : OFF_XTI + 256] = xTi.astype(bf)
    packed[:, OFF_WETS : OFF_WETS + 512] = wets2
    packed[:, OFF_WE : OFF_WE + 256] = we
    packed[:, OFF_MB : OFF_MB + 256] = mb2.astype(bf)
    packed[:, OFF_BIAS:] = biases

    ec = np.asarray(edge_attr[b, i0 : i0 + P], np.float32)  # [128 i, 256 j, 64]
    en_h = (ec.transpose(1, 0, 2).reshape(2, P, P * DE)).astype(bf)
    eT2_h = np.ascontiguousarray(
        ec.reshape(64, 2, 2, 128, DE).transpose(2, 1, 4, 0, 3)
    ).reshape(2, P, 64 * P).astype(f8)
    return {
        "packed": packed,
        "en": np.ascontiguousarray(en_h),
        "eT2": np.ascontiguousarray(eT2_h),
    }


def kernel(x, edge_attr, attn_mask, W_q, b_q, W_k, b_k, W_v, b_v, W_e, W_s, b_s):
    from concourse.bass_utils import run_bass_kernel_spmd

    x = np.asarray(x, dtype=np.float32)
    edge_attr = np.asarray(edge_attr, dtype=np.float32)
    attn_mask = np.asarray(attn_mask)
    wblk = _prep_weights(W_q, b_q, W_k, b_k, W_v, b_v, W_e, W_s, b_s)

    nc = _get_program(wblk[-1])
    in_maps = [
        _prep_core_inputs(c, x, edge_attr, attn_mask, wblk) for c in range(NCORES)
    ]
    res = run_bass_kernel_spmd(nc, in_maps, core_ids=list(range(NCORES)))
    outv = np.empty((B, N, D), dtype=np.float32)
    for c in range(NCORES):
        b, ih = c // 2, c % 2
        outv[b, ih * P : (ih + 1) * P] = np.asarray(res.results[c]["out"])
    return outv
